# revision 6
# baseline (speedup 1.0000x reference)
"""GAT layer (nn_GAT_40037685133531) as a Trainium2 Bass kernel on 8 NeuronCores.

v2 strategy (destination-lane, gather-bulk):
  - Destination nodes sharded 8 ways; within a core, nodes are ordered by
    lex(max(lo_deg, 3*hi_deg), lo_deg) and assigned to (tile, lane) so each
    tile's 128 lanes have near-uniform slot counts (row padding ~1.19x).
    Aggregation is a pairwise fold over each lane's slots — no one-hot
    scatter matmuls at all.
  - Phase 0 (replicated): htab[row] = [h (c-major, 128) | a_s (4) | a_d (4)]
    bf16 in 512B rows from x_T-chunk matmuls against
    W_ext = [W*Pcmaj | W@A_s | W@A_d]; PSUM->bf16 cast on the Scalar engine.
  - Rows are a per-core permutation: own nodes at rows 0..6271 (a_d then
    comes from one direct batched DMA), high-out-degree others in the
    int16-addressable lo half, rest in the hi half (dma_gather indices are
    signed int16 — HW-probed). Dummy rows built from a "magic" x column with
    x @ (W@A_s) = -3e4 make padded slots exp to 0 with no masking.
  - Phase 1 gathers each tile's edge rows with gpsimd.dma_gather in chunks
    of <=1024 indices (HW limit) round-robined over 4 SWDGE queues (the
    gather path is DMA-bandwidth-bound at ~107GB/s per queue).
    ex = exp(leakyrelu(a_s + a_d)) (Exp on the Scalar engine), messages
    [ex*h | ex] fold pairwise in bf16, normalize + ELU + (PE transpose,
    z @ W2) finish the tile; y writes are batched 7 tiles per DMA.
"""

import os
import sys

import numpy as np

if "/opt/trn_rl_repo" not in sys.path:
    sys.path.insert(0, "/opt/trn_rl_repo")

import ml_dtypes

N_NODES = 50000
N_EDGES = 800000
F_IN = 128
HEADS = 4
HIDDEN = 32
F_OUT = 64
NEG = 0.2
N_CORES = 8
P = 128
NPC = N_NODES // N_CORES            # 6250
T_TILES = (NPC + P - 1) // P        # 49
LANES = T_TILES * P                 # 6272
N_ROWS = 50304                      # 393*128 >= 6272 + 43750 + dummies
LO_CAP = 32768
LO_DUMMY = LO_CAP - 1               # row 32767
HI_DUMMY = N_ROWS - 1               # row 50303
OTH_LO = LO_DUMMY - LANES           # 26495 "other" nodes in the lo half
H8 = os.environ.get("GAT_H8", "0") == "1"   # fp8 h rows (256B) vs bf16 (512B)
EB = 256                            # elems per htab row (512B bf16 / 256B u8)
ROW_USED = F_IN + 2 * HEADS         # 136 (bf16 row layout)
RB_H = F_IN                         # u8 row: bytes 0:128   h fp8
RB_A = F_IN + 2 * 2 * HEADS         # u8 row: bytes 128:144 a_s|a_d bf16
MAGIC = -300.0
CHUNK = 1024                        # dma_gather num_idxs limit (HW-probed)
P0_CH = 3                           # node tiles per PSUM bank
P0_GRP = 4                          # PSUM tiles per htab write
P0_CHUNKS = N_ROWS // (P0_CH * P)   # 131 (odd -> last group is single)
YB = 7                              # tiles per y write
NQ = 4                              # SWDGE queues

PERM_CMAJ = (np.arange(F_IN) % HEADS) * HIDDEN + np.arange(F_IN) // HEADS


def _chunks(n):
    out = []
    while n > 0:
        c = min(CHUNK, n)
        out.append(c)
        n -= c
    return out


def _align32(c):
    return (c + 31) // 32 * 32


def _sched(K_lo, K_hi):
    """Per-tile gather calls: list of (is_hi, slot0, nrows, col0).
    col0 is 32-col (64B) aligned for direct idxs_ap slicing."""
    tiles = []
    col = 0
    for t in range(T_TILES):
        items = []
        s0 = 0
        for sz in _chunks(K_lo[t] * P):
            col = _align32(col)
            items.append((False, s0, sz, col))
            s0 += sz // P
            col += sz // 16
        for sz in _chunks(K_hi[t] * P):
            col = _align32(col)
            items.append((True, s0, sz, col))
            s0 += sz // P
            col += sz // 16
        tiles.append(items)
    return tiles, _align32(col)


def _wrap16(a):
    n = len(a)
    assert n % 16 == 0
    return np.tile(np.asarray(a, np.int16).reshape(n // 16, 16).T, (8, 1))


def _prep(x, edge_index, W, a_src, a_dst):
    src = np.ascontiguousarray(np.asarray(edge_index[0]).astype(np.int64))
    dst = np.ascontiguousarray(np.asarray(edge_index[1]).astype(np.int64))
    x = np.asarray(x, np.float32)
    W = np.asarray(W, np.float32)

    A_s = np.zeros((F_IN, HEADS), np.float32)
    A_d = np.zeros((F_IN, HEADS), np.float32)
    for h in range(HEADS):
        A_s[h * HIDDEN:(h + 1) * HIDDEN, h] = a_src[h]
        A_d[h * HIDDEN:(h + 1) * HIDDEN, h] = a_dst[h]
    WA_s = W @ A_s
    WA_d = W @ A_d
    # fp8 rows store h/4 (keeps the magic-dummy h inside e4m3 range)
    hscale = 0.25 if H8 else 1.0
    W_ext = np.concatenate([W[:, PERM_CMAJ] * hscale, WA_s, WA_d], axis=1)

    # magic dummy column: x_m @ WA_s = -3e4 per head -> padded slots exp to 0
    xmagic = np.linalg.lstsq(WA_s.T, np.full(HEADS, MAGIC, np.float32),
                             rcond=None)[0].astype(np.float32)
    assert np.all(xmagic @ WA_s < 0.8 * MAGIC)
    if H8:
        assert np.max(np.abs(xmagic @ W)) * hscale < 200.0  # fp8-safe dummy

    indeg = np.bincount(dst, minlength=N_NODES)
    outcnt = np.bincount(src, minlength=N_NODES)

    cores = []
    for c in range(N_CORES):
        own = np.arange(c * NPC, (c + 1) * NPC)
        em = (dst >= c * NPC) & (dst < (c + 1) * NPC)
        es, ed = src[em], dst[em]
        refcnt = np.bincount(es, minlength=N_NODES)
        oth_mask = np.ones(N_NODES, bool)
        oth_mask[own] = False
        others = np.where(oth_mask & (refcnt > 0))[0]
        oth_order = others[np.argsort(-refcnt[others], kind="stable")]
        lo_oth = oth_order[:OTH_LO]
        hi_oth = oth_order[OTH_LO:]
        lo_set = np.zeros(N_NODES, bool)
        lo_set[lo_oth] = True
        lo_set[own] = True
        eln = ed - c * NPC
        in_lo = lo_set[es]
        nlo_n = np.bincount(eln[in_lo], minlength=NPC)
        nhi_n = np.bincount(eln[~in_lo], minlength=NPC)
        order = np.lexsort((nlo_n, np.maximum(nlo_n, 3 * nhi_n)))[::-1]
        own_sorted = own[order]

        rank = np.full(N_NODES, -1, np.int64)
        rank[own_sorted] = np.arange(NPC)
        row_of = np.full(N_NODES, -1, np.int64)
        row_of[own_sorted] = np.arange(NPC)
        row_of[lo_oth] = LANES + np.arange(len(lo_oth))
        row_of[hi_oth] = LO_CAP + 1 + np.arange(len(hi_oth))
        n_comp = LO_CAP + 1 + len(hi_oth)

        xT = np.zeros((F_IN, N_ROWS), np.float32)
        has_row = row_of >= 0
        xT[:, row_of[has_row]] = x.T[:, has_row]
        xT[:, LO_DUMMY] = xmagic
        xT[:, LO_CAP] = xmagic      # hi dummy = first hi row

        lr = rank[ed]
        srow = row_of[es]
        hf = (srow >= LO_CAP).astype(np.int64)
        o = np.lexsort((hf, lr))
        lrs, srs, hfs = lr[o], srow[o], hf[o]
        counts = np.bincount(lrs, minlength=LANES)
        starts = np.zeros(LANES + 1, np.int64)
        starts[1:] = np.cumsum(counts)
        cc = np.arange(len(lrs)) - starts[lrs]
        nlo = np.bincount(lrs[hfs == 0], minlength=LANES)
        slot = np.where(hfs == 0, cc, cc - nlo[lrs])
        cores.append(dict(
            own_sorted=own_sorted, xT=xT, n_comp=n_comp,
            lrs=lrs, srs=srs, hfs=hfs, slot=slot,
            K_lo=nlo.reshape(T_TILES, P).max(1),
            K_hi=(counts - nlo).reshape(T_TILES, P).max(1),
        ))

    K_lo = np.maximum(np.max([cr["K_lo"] for cr in cores], axis=0), 1)
    K_hi = np.max([cr["K_hi"] for cr in cores], axis=0)

    lo_off = np.zeros(T_TILES + 1, np.int64)
    lo_off[1:] = np.cumsum(K_lo * P)
    hi_off = np.zeros(T_TILES + 1, np.int64)
    hi_off[1:] = np.cumsum(K_hi * P)

    sched, cols = _sched(K_lo, K_hi)

    for cr in cores:
        lrs, srs, hfs, slot = cr["lrs"], cr["srs"], cr["hfs"], cr["slot"]
        tl, ln = lrs // P, lrs % P
        flat_lo = np.full(lo_off[-1], LO_DUMMY, np.int16)
        m = hfs == 0
        flat_lo[lo_off[tl[m]] + slot[m] * P + ln[m]] = srs[m].astype(np.int16)
        flat_hi = np.zeros(hi_off[-1], np.int16)  # pad -> row LO_CAP (dummy)
        m = hfs == 1
        flat_hi[hi_off[tl[m]] + slot[m] * P + ln[m]] = (
            srs[m] - LO_CAP).astype(np.int16)

        idx = np.zeros((P, cols), np.int16)
        for t in range(T_TILES):
            for is_hi, s0, nrows, col0 in sched[t]:
                if is_hi:
                    base = hi_off[t] + (s0 - K_lo[t]) * P
                    seg = flat_hi[base:base + nrows]
                else:
                    base = lo_off[t] + s0 * P
                    seg = flat_lo[base:base + nrows]
                idx[:, col0:col0 + nrows // 16] = _wrap16(seg)
        cr["idx"] = np.ascontiguousarray(idx)

    grain = P0_CH * P
    n_comp = (max(cr["n_comp"] for cr in cores) + grain - 1) // grain * grain
    return (W_ext, cores, tuple(K_lo.tolist()), tuple(K_hi.tolist()), cols,
            n_comp)


def _build_module(K_lo, K_hi, cols, bias_nz, b2_nz, sim_safe,
                  n_comp):
    import concourse.mybir as mybir
    import concourse.tile as tile
    from concourse import bacc
    from concourse.masks import make_identity

    f32 = mybir.dt.float32
    bf16 = mybir.dt.bfloat16
    i16 = mybir.dt.int16
    u8 = mybir.dt.uint8
    fp8 = mybir.dt.float8e4
    add = mybir.AluOpType.add
    mult = mybir.AluOpType.mult
    amax = mybir.AluOpType.max
    Exp = mybir.ActivationFunctionType.Exp
    Copy = mybir.ActivationFunctionType.Copy
    Relu = mybir.ActivationFunctionType.Relu
    Recip = mybir.ActivationFunctionType.Reciprocal

    sched, cols2 = _sched(K_lo, K_hi)
    assert cols2 == cols

    nc = bacc.Bacc("TRN2", target_bir_lowering=False, debug=False,
                   num_devices=N_CORES, num_swdge_queues=NQ)

    x_T = nc.dram_tensor("x_T", [P, N_ROWS], bf16, kind="ExternalInput")
    W_ext_d = nc.dram_tensor("W_ext", [P, ROW_USED], bf16,
                             kind="ExternalInput")
    W2_d = nc.dram_tensor("W2", [P, F_OUT], bf16, kind="ExternalInput")
    idx_d = nc.dram_tensor("idx", [P, cols], i16, kind="ExternalInput")
    if bias_nz:
        bias_d = nc.dram_tensor("bias_row", [1, F_IN], f32,
                                kind="ExternalInput")
    if b2_nz:
        b2_d = nc.dram_tensor("b2_row", [1, F_OUT], f32, kind="ExternalInput")
    y_d = nc.dram_tensor("y_out", [LANES, F_OUT], f32, kind="ExternalOutput")
    tdt = u8 if H8 else bf16
    htab_lo = nc.dram_tensor("htab_lo", [LO_CAP, EB], tdt, kind="Internal")
    htab_hi = nc.dram_tensor("htab_hi", [N_ROWS - LO_CAP, EB], tdt,
                             kind="Internal")

    qn = [0]

    def next_q():
        q = qn[0]
        qn[0] = (q + 1) % NQ
        return q

    with tile.TileContext(nc) as tc:
        with tc.tile_pool(name="const", bufs=1) as constp:
            W_ext_sb = constp.tile([P, ROW_USED], bf16)
            nc.sync.dma_start(W_ext_sb[:], W_ext_d.ap())
            W2_sb = constp.tile([P, F_OUT], bf16)
            nc.sync.dma_start(W2_sb[:], W2_d.ap())
            idx_sb = constp.tile([P, cols], i16)
            nc.sync.dma_start(idx_sb[:], idx_d.ap())
            ident = constp.tile([P, P], f32)
            make_identity(nc, ident[:])
            adt_all = constp.tile([P, T_TILES * HEADS], bf16)
            if bias_nz or b2_nz:
                ones_sb = constp.tile([1, P], f32)
                nc.vector.memset(ones_sb[:], 1.0)
            if bias_nz:
                bias_row_sb = constp.tile([1, F_IN], f32)
                nc.sync.dma_start(bias_row_sb[:], bias_d.ap())
                bias_rep = constp.tile([P, F_IN], f32)
            if b2_nz:
                b2_sb = constp.tile([1, F_OUT], f32)
                nc.sync.dma_start(b2_sb[:], b2_d.ap())

            # ---- phase 0: htab rows = [h_cmaj | a_s | a_d] ----
            with (
                tc.tile_pool(name="xfull", bufs=1) as xfp,
                tc.tile_pool(name="hx", bufs=4) as hxp,
                tc.tile_pool(name="p0ps", bufs=6, space="PSUM") as p0ps,
            ):
                NC_ROWS = (P0_CHUNKS * P0_CH * P if sim_safe
                           else n_comp)
                xt = xfp.tile([P, NC_ROWS], bf16)
                NX = 8
                XSPL = NC_ROWS // NX // (P0_CH * P) * (P0_CH * P)
                xbounds = [i * XSPL for i in range(NX)] + [NC_ROWS]
                xengs = [nc.sync, nc.scalar]
                for xi in range(NX):
                    xengs[xi % 2].dma_start(
                        xt[:, xbounds[xi]:xbounds[xi + 1]],
                        x_T.ap()[:, xbounds[xi]:xbounds[xi + 1]])
                if bias_nz:
                    bps = p0ps.tile([P, F_IN], f32)
                    nc.tensor.matmul(bps[:], lhsT=ones_sb[:],
                                     rhs=bias_row_sb[:], start=True, stop=True)
                    nc.vector.tensor_copy(bias_rep[:], bps[:])
                RU = RB_A if H8 else ROW_USED   # table elems used per row
                NTL = NC_ROWS // P               # total node tiles computed
                LO_T = LO_CAP // P               # 256 tiles in the lo table
                ti = 0
                while ti < NTL:
                    seg_end = LO_T if ti < LO_T else NTL
                    nt = min(P0_GRP * P0_CH, seg_end - ti)
                    c0 = ti * P
                    hx = hxp.tile([P, P0_GRP * P0_CH * RU],
                                  u8 if H8 else bf16, tag="hx")
                    hx3 = hx[:, 0:nt * RU].rearrange("p (t e) -> p t e", t=nt)
                    gi = 0
                    while gi < nt:
                        ng = min(P0_CH, nt - gi)
                        ps = p0ps.tile([P, P0_CH * ROW_USED], f32, tag="ps")
                        for j in range(ng):
                            jj = gi + j
                            nc.tensor.matmul(
                                ps[:, j * ROW_USED:(j + 1) * ROW_USED],
                                lhsT=xt[:, c0 + jj * P:c0 + (jj + 1) * P],
                                rhs=W_ext_sb[:], start=True, stop=True)
                        ps3 = (ps[:, 0:ng * ROW_USED]
                               .rearrange("p (t e) -> p t e", t=ng))
                        gs = slice(gi, gi + ng)
                        if H8:
                            nc.scalar.activation(
                                out=hx3[:, gs, 0:F_IN].bitcast(fp8),
                                in_=ps3[:, :, 0:F_IN], func=Copy)
                            nc.scalar.activation(
                                out=hx3[:, gs, F_IN:RB_A].bitcast(bf16),
                                in_=ps3[:, :, F_IN:ROW_USED], func=Copy)
                        elif (gi // P0_CH) % 2 == 0:
                            nc.scalar.activation(
                                out=hx3[:, gs, :], in_=ps3, func=Copy)
                        else:
                            nc.vector.tensor_copy(hx3[:, gs, :], ps3)
                        gi += ng
                    if ti < LO_T:
                        dst = htab_lo.ap()[c0:c0 + nt * P, 0:RU]
                    else:
                        dst = htab_hi.ap()[c0 - LO_CAP:c0 - LO_CAP + nt * P,
                                           0:RU]
                    nc.sync.dma_start(
                        dst.rearrange("(t p) e -> p t e", p=P), hx3)
                    ti += nt
                if sim_safe:
                    # sim's NaN canary: initialize the tables' row padding
                    zpad = xfp.tile([P, EB], u8 if H8 else bf16)
                    nc.gpsimd.memset(zpad[:], 0)
                    for tens, nrows in ((htab_lo, LO_CAP),
                                        (htab_hi, N_ROWS - LO_CAP)):
                        for r0 in range(0, nrows, P):
                            nc.sync.dma_start(
                                tens.ap()[r0:r0 + P, RU:]
                                .rearrange("(t p) e -> p t e", p=P),
                                zpad[:, 0:EB - RU].unsqueeze(1)
                                .to_broadcast([P, 1, EB - RU]))

            # batched a_d for all tiles: adt_all[p, t*4:(t+1)*4]
            if H8:
                ad_src = (htab_lo.ap()[0:LANES, F_IN + 2 * HEADS:RB_A]
                          .bitcast(bf16))
            else:
                ad_src = htab_lo.ap()[0:LANES,
                                      F_IN + HEADS:F_IN + 2 * HEADS]
            nc.sync.dma_start(
                adt_all[:].rearrange("p (t h) -> p t h", t=T_TILES),
                ad_src.rearrange("(t p) h -> p t h", p=P))

            # ---- phase 1: per destination tile ----
            with (
                tc.tile_pool(name="g", bufs=4) as gp,
                tc.tile_pool(name="mb", bufs=2) as mbp,
                tc.tile_pool(name="small", bufs=2) as smallp,
                tc.tile_pool(name="ys", bufs=2) as ysp,
                tc.tile_pool(name="pt", bufs=2, space="PSUM") as ptp,
                tc.tile_pool(name="yp", bufs=2, space="PSUM") as ypp,
            ):
                ysb = None
                for t in range(T_TILES):
                    K = K_lo[t] + K_hi[t]
                    g = gp.tile([P, K * EB], u8 if H8 else bf16, tag="g")
                    g3 = g[:].rearrange("p (k e) -> p k e", k=K)
                    if H8:
                        gv_h = g3[:, :, 0:F_IN].bitcast(fp8)
                        gv_as = (g3[:, :, F_IN:RB_A]
                                 .bitcast(bf16)[:, :, 0:HEADS])
                    else:
                        gv_h = g3[:, :, 0:F_IN]
                        gv_as = g3[:, :, F_IN:F_IN + HEADS]
                    for is_hi, s0, nrows, col0 in sched[t]:
                        nc.gpsimd.dma_gather(
                            out_ap=g3[:, s0:s0 + nrows // P, :],
                            in_ap=(htab_hi.ap() if is_hi
                                   else htab_lo.ap()),
                            idxs_ap=idx_sb[:, col0:col0 + nrows // 16],
                            num_idxs=nrows, num_idxs_reg=nrows,
                            elem_size=EB, queue_num=next_q())

                    exb = smallp.tile([P, K * HEADS], bf16, tag="exb")
                    ex3 = exb[:].rearrange("p (k h) -> p k h", k=K)
                    nc.vector.tensor_tensor(
                        out=ex3, in0=gv_as,
                        in1=adt_all[:, t * HEADS:(t + 1) * HEADS]
                        .unsqueeze(1).to_broadcast([P, K, HEADS]),
                        op=add)
                    nc.vector.scalar_tensor_tensor(
                        out=exb[:], in0=exb[:], scalar=NEG, in1=exb[:],
                        op0=mult, op1=amax)
                    mb = mbp.tile([P, K * 132], bf16, tag="mb")
                    m3 = mb[:].rearrange("p (k f) -> p k f", k=K)
                    nc.scalar.activation(out=m3[:, :, F_IN:132], in_=ex3,
                                         func=Exp)
                    nc.vector.tensor_tensor(
                        out=m3[:, :, 0:F_IN].rearrange(
                            "p k (c h) -> p k c h", h=HEADS),
                        in0=gv_h.rearrange(
                            "p k (c h) -> p k c h", h=HEADS),
                        in1=m3[:, :, F_IN:132].unsqueeze(2).to_broadcast(
                            [P, K, HIDDEN, HEADS]),
                        op=mult)

                    accf = smallp.tile([P, 132], f32, tag="accf")
                    kk = K
                    while kk > 2:
                        pr = kk // 2
                        nc.vector.tensor_tensor(
                            out=mb[:, 0:pr * 132], in0=mb[:, 0:pr * 132],
                            in1=mb[:, (kk - pr) * 132:kk * 132], op=add)
                        kk -= pr
                    if kk == 2:
                        # fold + eps in one op (adding eps to h cols is benign)
                        nc.vector.scalar_tensor_tensor(
                            out=accf[:], in0=mb[:, 0:132], scalar=1e-16,
                            in1=mb[:, 132:264], op0=add, op1=add)
                    else:
                        nc.vector.tensor_scalar_add(out=accf[:],
                                                    in0=mb[:, 0:132],
                                                    scalar1=1e-16)

                    rec = smallp.tile([P, HEADS], f32, tag="rec")
                    nc.vector.reciprocal(rec[:], accf[:, F_IN:132])
                    zn = smallp.tile([P, F_IN], f32, tag="zn")
                    nc.vector.scalar_tensor_tensor(
                        out=zn[:].rearrange("p (c h) -> p c h", h=HEADS),
                        in0=accf[:, 0:F_IN].rearrange("p (c h) -> p c h",
                                                      h=HEADS),
                        scalar=4.0 if H8 else 1.0,
                        in1=rec[:].unsqueeze(1).to_broadcast(
                            [P, HIDDEN, HEADS]),
                        op0=mult, op1=mult)
                    if bias_nz:
                        nc.vector.tensor_tensor(out=zn[:], in0=zn[:],
                                                in1=bias_rep[:], op=add)
                    # ELU(z) = max(z, exp(-Relu(-z)) - 1)
                    tmp = smallp.tile([P, F_IN], f32, tag="tmp")
                    nc.scalar.activation(out=tmp[:], in_=zn[:], func=Relu,
                                         scale=-1.0)
                    nc.scalar.activation(out=tmp[:], in_=tmp[:], func=Exp,
                                         scale=-1.0)
                    nc.vector.scalar_tensor_tensor(
                        out=zn[:], in0=tmp[:], scalar=-1.0, in1=zn[:],
                        op0=add, op1=amax)

                    pt = ptp.tile([P, P], f32, tag="pt")
                    nc.tensor.transpose(out=pt[:], in_=zn[:],
                                        identity=ident[:])
                    znT = smallp.tile([P, P], bf16, tag="znT")
                    nc.scalar.activation(out=znT[:], in_=pt[:], func=Copy)
                    yp = ypp.tile([P, F_OUT], f32, tag="yp")
                    nc.tensor.matmul(yp[:], lhsT=znT[:], rhs=W2_sb[:],
                                     start=True, stop=not b2_nz)
                    if b2_nz:
                        nc.tensor.matmul(yp[:], lhsT=ones_sb[:], rhs=b2_sb[:],
                                         start=False, stop=True)
                    if t % YB == 0:
                        ysb = ysp.tile([P, YB * F_OUT], f32, tag="ysb")
                    nc.scalar.activation(
                        out=ysb[:, (t % YB) * F_OUT:(t % YB + 1) * F_OUT],
                        in_=yp[:], func=Copy)
                    if t % YB == YB - 1:
                        t0 = t - (YB - 1)
                        nc.sync.dma_start(
                            y_d.ap()[t0 * P:(t + 1) * P, :]
                            .rearrange("(t p) f -> p t f", p=P),
                            ysb[:].rearrange("p (t f) -> p t f", t=YB))

    nc.compile()
    return nc


_MODULE_CACHE = {}


def _get_module(K_lo, K_hi, cols, bias_nz, b2_nz, sim_safe, n_comp):
    key = (K_lo, K_hi, cols, bias_nz, b2_nz, sim_safe, H8, n_comp)
    if key not in _MODULE_CACHE:
        _MODULE_CACHE[key] = _build_module(K_lo, K_hi, cols, bias_nz, b2_nz,
                                           sim_safe, n_comp)
    return _MODULE_CACHE[key]


def _ensure_ntff_hook():
    """The axon NTFF profile hook lives in antenv.axon_hooks, which this
    image's antenv package lacks; shim it so trace=True works."""
    try:
        import antenv.axon_hooks  # noqa: F401
        return
    except ImportError:
        pass
    import types

    import antenv

    mod = types.ModuleType("antenv.axon_hooks")
    holder = {"h": None}
    mod.set_axon_ntff_profile_hook = lambda h: holder.__setitem__("h", h)
    mod.get_axon_ntff_profile_hook = lambda: holder["h"]
    try:
        from trn_agent_boot.trn_boot import _ntff_profile_via_ctypes
        holder["h"] = _ntff_profile_via_ctypes("/opt/axon/libaxon_pjrt.so")
    except Exception:
        pass
    sys.modules["antenv.axon_hooks"] = mod
    antenv.axon_hooks = mod


def kernel(x, edge_index, edge_weight, W, a_src, a_dst, bias, W2, b2,
           _trace=False, _sim=False):
    from concourse.bass_utils import run_bass_kernel_spmd

    if _trace:
        _ensure_ntff_hook()

    x = np.ascontiguousarray(np.asarray(x, np.float32))
    W = np.asarray(W, np.float32)
    a_src = np.asarray(a_src, np.float32)
    a_dst = np.asarray(a_dst, np.float32)
    bias = np.asarray(bias, np.float32)
    W2 = np.ascontiguousarray(np.asarray(W2, np.float32))
    b2 = np.asarray(b2, np.float32)

    W_ext, cores, K_lo, K_hi, cols, n_comp = _prep(x, edge_index, W, a_src,
                                                   a_dst)
    bias_nz = bool(np.any(bias))
    b2_nz = bool(np.any(b2))
    nc = _get_module(K_lo, K_hi, cols, bias_nz, b2_nz, _sim, n_comp)

    bf = ml_dtypes.bfloat16
    W_ext_bf = W_ext.astype(bf)
    W2_bf = W2[PERM_CMAJ, :].astype(bf)

    in_maps = []
    for cr in cores:
        m = {
            "x_T": cr["xT"].astype(bf),
            "W_ext": W_ext_bf,
            "W2": W2_bf,
            "idx": cr["idx"],
        }
        if bias_nz:
            m["bias_row"] = np.ascontiguousarray(
                bias[PERM_CMAJ].reshape(1, F_IN))
        if b2_nz:
            m["b2_row"] = b2.reshape(1, F_OUT)
        in_maps.append(m)

    if _sim:
        from concourse.bass_interp import CoreSim
        sim = CoreSim(nc, trace=False)
        for k, v in in_maps[0].items():
            sim.tensor(k)[:] = v
        sim.simulate()
        y0 = np.asarray(sim.tensor("y_out"), np.float32)
        out = np.zeros((N_NODES, F_OUT), np.float32)
        out[cores[0]["own_sorted"]] = y0[:NPC]
        kernel.sim_out = out
        return out

    res = run_bass_kernel_spmd(nc, in_maps, core_ids=list(range(N_CORES)),
                               trace=_trace)
    out = np.zeros((N_NODES, F_OUT), np.float32)
    for c, cr in enumerate(cores):
        out[cr["own_sorted"]] = np.asarray(
            res.results[c]["y_out"], np.float32)[:NPC]
    if _trace:
        kernel.last_results = res
    return out


# revision 7
# speedup vs baseline: 1.8622x; 1.8622x over previous
"""GAT layer (nn_GAT_40037685133531) as a Trainium2 Bass kernel on 8 NeuronCores.

v2 strategy (destination-lane, gather-bulk):
  - Destination nodes sharded 8 ways; within a core, nodes are ordered by
    lex(max(lo_deg, 3*hi_deg), lo_deg) and assigned to (tile, lane) so each
    tile's 128 lanes have near-uniform slot counts (row padding ~1.19x).
    Aggregation is a pairwise fold over each lane's slots — no one-hot
    scatter matmuls at all.
  - Phase 0 (replicated): htab[row] = [h (c-major, 128) | a_s (4) | a_d (4)]
    bf16 in 512B rows from x_T-chunk matmuls against
    W_ext = [W*Pcmaj | W@A_s | W@A_d]; PSUM->bf16 cast on the Scalar engine.
  - Rows are a per-core permutation: own nodes at rows 0..6271 (a_d then
    comes from one direct batched DMA), high-out-degree others in the
    int16-addressable lo half, rest in the hi half (dma_gather indices are
    signed int16 — HW-probed). Dummy rows built from a "magic" x column with
    x @ (W@A_s) = -3e4 make padded slots exp to 0 with no masking.
  - Phase 1 gathers each tile's edge rows with gpsimd.dma_gather in chunks
    of <=1024 indices (HW limit) round-robined over 4 SWDGE queues (the
    gather path is DMA-bandwidth-bound at ~107GB/s per queue).
    ex = exp(leakyrelu(a_s + a_d)) (Exp on the Scalar engine), messages
    [ex*h | ex] fold pairwise in bf16, normalize + ELU + (PE transpose,
    z @ W2) finish the tile; y writes are batched 7 tiles per DMA.
"""

import os
import sys

import numpy as np

if "/opt/trn_rl_repo" not in sys.path:
    sys.path.insert(0, "/opt/trn_rl_repo")

import ml_dtypes

N_NODES = 50000
N_EDGES = 800000
F_IN = 128
HEADS = 4
HIDDEN = 32
F_OUT = 64
NEG = 0.2
N_CORES = 8
P = 128
NPC = N_NODES // N_CORES            # 6250
T_TILES = (NPC + P - 1) // P        # 49
LANES = T_TILES * P                 # 6272
N_ROWS = 50304                      # 393*128 >= 6272 + 43750 + dummies
LO_CAP = 32768
LO_DUMMY = LO_CAP - 1               # row 32767
HI_DUMMY = N_ROWS - 1               # row 50303
OTH_LO = LO_DUMMY - LANES           # 26495 "other" nodes in the lo half
H8 = os.environ.get("GAT_H8", "0") == "1"   # fp8 h rows (256B) vs bf16 (512B)
EB = 256                            # elems per htab row (512B bf16 / 256B u8)
ROW_USED = F_IN + 2 * HEADS         # 136 (bf16 row layout)
RB_H = F_IN                         # u8 row: bytes 0:128   h fp8
RB_A = F_IN + 2 * 2 * HEADS         # u8 row: bytes 128:144 a_s|a_d bf16
MAGIC = -300.0
CHUNK = 1024                        # dma_gather num_idxs limit (HW-probed)
P0_CH = 3                           # node tiles per PSUM bank
P0_GRP = 4                          # PSUM tiles per htab write
P0_CHUNKS = N_ROWS // (P0_CH * P)   # 131 (odd -> last group is single)
YB = 7                              # tiles per y write
NQ = 4                              # SWDGE queues

PERM_CMAJ = (np.arange(F_IN) % HEADS) * HIDDEN + np.arange(F_IN) // HEADS


def _chunks(n):
    out = []
    while n > 0:
        c = min(CHUNK, n)
        out.append(c)
        n -= c
    return out


def _align32(c):
    return (c + 31) // 32 * 32


def _sched(K_lo, K_hi):
    """Per-tile gather calls: list of (is_hi, slot0, nrows, col0).
    col0 is 32-col (64B) aligned for direct idxs_ap slicing."""
    tiles = []
    col = 0
    for t in range(T_TILES):
        items = []
        s0 = 0
        for sz in _chunks(K_lo[t] * P):
            col = _align32(col)
            items.append((False, s0, sz, col))
            s0 += sz // P
            col += sz // 16
        for sz in _chunks(K_hi[t] * P):
            col = _align32(col)
            items.append((True, s0, sz, col))
            s0 += sz // P
            col += sz // 16
        tiles.append(items)
    return tiles, _align32(col)


def _wrap16(a):
    n = len(a)
    assert n % 16 == 0
    return np.tile(np.asarray(a, np.int16).reshape(n // 16, 16).T, (8, 1))


def _prep(x, edge_index, W, a_src, a_dst):
    src = np.ascontiguousarray(np.asarray(edge_index[0]).astype(np.int64))
    dst = np.ascontiguousarray(np.asarray(edge_index[1]).astype(np.int64))
    x = np.asarray(x, np.float32)
    W = np.asarray(W, np.float32)

    A_s = np.zeros((F_IN, HEADS), np.float32)
    A_d = np.zeros((F_IN, HEADS), np.float32)
    for h in range(HEADS):
        A_s[h * HIDDEN:(h + 1) * HIDDEN, h] = a_src[h]
        A_d[h * HIDDEN:(h + 1) * HIDDEN, h] = a_dst[h]
    WA_s = W @ A_s
    WA_d = W @ A_d
    # fp8 rows store h/4 (keeps the magic-dummy h inside e4m3 range)
    hscale = 0.25 if H8 else 1.0
    W_ext = np.concatenate([W[:, PERM_CMAJ] * hscale, WA_s, WA_d], axis=1)

    # magic dummy column: x_m @ WA_s = -3e4 per head -> padded slots exp to 0
    xmagic = np.linalg.lstsq(WA_s.T, np.full(HEADS, MAGIC, np.float32),
                             rcond=None)[0].astype(np.float32)
    assert np.all(xmagic @ WA_s < 0.8 * MAGIC)
    if H8:
        assert np.max(np.abs(xmagic @ W)) * hscale < 200.0  # fp8-safe dummy

    indeg = np.bincount(dst, minlength=N_NODES)
    outcnt = np.bincount(src, minlength=N_NODES)

    cores = []
    for c in range(N_CORES):
        own = np.arange(c * NPC, (c + 1) * NPC)
        em = (dst >= c * NPC) & (dst < (c + 1) * NPC)
        es, ed = src[em], dst[em]
        refcnt = np.bincount(es, minlength=N_NODES)
        oth_mask = np.ones(N_NODES, bool)
        oth_mask[own] = False
        others = np.where(oth_mask & (refcnt > 0))[0]
        oth_order = others[np.argsort(-refcnt[others], kind="stable")]
        lo_oth = oth_order[:OTH_LO]
        hi_oth = oth_order[OTH_LO:]
        lo_set = np.zeros(N_NODES, bool)
        lo_set[lo_oth] = True
        lo_set[own] = True
        eln = ed - c * NPC
        in_lo = lo_set[es]
        nlo_n = np.bincount(eln[in_lo], minlength=NPC)
        nhi_n = np.bincount(eln[~in_lo], minlength=NPC)
        order = np.lexsort((nlo_n, np.maximum(nlo_n, 3 * nhi_n)))[::-1]
        own_sorted = own[order]

        rank = np.full(N_NODES, -1, np.int64)
        rank[own_sorted] = np.arange(NPC)
        row_of = np.full(N_NODES, -1, np.int64)
        row_of[own_sorted] = np.arange(NPC)
        row_of[lo_oth] = LANES + np.arange(len(lo_oth))
        row_of[hi_oth] = LO_CAP + 1 + np.arange(len(hi_oth))
        n_comp = LO_CAP + 1 + len(hi_oth)

        xT = np.zeros((F_IN, N_ROWS), np.float32)
        has_row = row_of >= 0
        xT[:, row_of[has_row]] = x.T[:, has_row]
        xT[:, LO_DUMMY] = xmagic
        xT[:, LO_CAP] = xmagic      # hi dummy = first hi row

        lr = rank[ed]
        srow = row_of[es]
        hf = (srow >= LO_CAP).astype(np.int64)
        o = np.lexsort((hf, lr))
        lrs, srs, hfs = lr[o], srow[o], hf[o]
        counts = np.bincount(lrs, minlength=LANES)
        starts = np.zeros(LANES + 1, np.int64)
        starts[1:] = np.cumsum(counts)
        cc = np.arange(len(lrs)) - starts[lrs]
        nlo = np.bincount(lrs[hfs == 0], minlength=LANES)
        slot = np.where(hfs == 0, cc, cc - nlo[lrs])
        cores.append(dict(
            own_sorted=own_sorted, xT=xT, n_comp=n_comp,
            lrs=lrs, srs=srs, hfs=hfs, slot=slot,
            K_lo=nlo.reshape(T_TILES, P).max(1),
            K_hi=(counts - nlo).reshape(T_TILES, P).max(1),
        ))

    K_lo = np.maximum(np.max([cr["K_lo"] for cr in cores], axis=0), 1)
    K_hi = np.max([cr["K_hi"] for cr in cores], axis=0)

    lo_off = np.zeros(T_TILES + 1, np.int64)
    lo_off[1:] = np.cumsum(K_lo * P)
    hi_off = np.zeros(T_TILES + 1, np.int64)
    hi_off[1:] = np.cumsum(K_hi * P)

    sched, cols = _sched(K_lo, K_hi)

    for cr in cores:
        lrs, srs, hfs, slot = cr["lrs"], cr["srs"], cr["hfs"], cr["slot"]
        tl, ln = lrs // P, lrs % P
        flat_lo = np.full(lo_off[-1], LO_DUMMY, np.int16)
        m = hfs == 0
        flat_lo[lo_off[tl[m]] + slot[m] * P + ln[m]] = srs[m].astype(np.int16)
        flat_hi = np.zeros(hi_off[-1], np.int16)  # pad -> row LO_CAP (dummy)
        m = hfs == 1
        flat_hi[hi_off[tl[m]] + slot[m] * P + ln[m]] = (
            srs[m] - LO_CAP).astype(np.int16)

        idx = np.zeros((P, cols), np.int16)
        for t in range(T_TILES):
            for is_hi, s0, nrows, col0 in sched[t]:
                if is_hi:
                    base = hi_off[t] + (s0 - K_lo[t]) * P
                    seg = flat_hi[base:base + nrows]
                else:
                    base = lo_off[t] + s0 * P
                    seg = flat_lo[base:base + nrows]
                idx[:, col0:col0 + nrows // 16] = _wrap16(seg)
        cr["idx"] = np.ascontiguousarray(idx)

    grain = P0_CH * P
    n_comp = (max(cr["n_comp"] for cr in cores) + grain - 1) // grain * grain
    return (W_ext, cores, tuple(K_lo.tolist()), tuple(K_hi.tolist()), cols,
            n_comp)


def _build_module(K_lo, K_hi, cols, bias_nz, b2_nz, sim_safe,
                  n_comp):
    import concourse.mybir as mybir
    import concourse.tile as tile
    from concourse import bacc
    from concourse.masks import make_identity

    f32 = mybir.dt.float32
    bf16 = mybir.dt.bfloat16
    i16 = mybir.dt.int16
    u8 = mybir.dt.uint8
    fp8 = mybir.dt.float8e4
    add = mybir.AluOpType.add
    mult = mybir.AluOpType.mult
    amax = mybir.AluOpType.max
    Exp = mybir.ActivationFunctionType.Exp
    Copy = mybir.ActivationFunctionType.Copy
    Relu = mybir.ActivationFunctionType.Relu
    Recip = mybir.ActivationFunctionType.Reciprocal

    sched, cols2 = _sched(K_lo, K_hi)
    assert cols2 == cols

    nc = bacc.Bacc("TRN2", target_bir_lowering=False, debug=False,
                   num_devices=N_CORES, num_swdge_queues=NQ)

    x_T = nc.dram_tensor("x_T", [P, N_ROWS], bf16, kind="ExternalInput")
    W_ext_d = nc.dram_tensor("W_ext", [P, ROW_USED], bf16,
                             kind="ExternalInput")
    W2_d = nc.dram_tensor("W2", [P, F_OUT], bf16, kind="ExternalInput")
    idx_d = nc.dram_tensor("idx", [P, cols], i16, kind="ExternalInput")
    if bias_nz:
        bias_d = nc.dram_tensor("bias_row", [1, F_IN], f32,
                                kind="ExternalInput")
    if b2_nz:
        b2_d = nc.dram_tensor("b2_row", [1, F_OUT], f32, kind="ExternalInput")
    y_d = nc.dram_tensor("y_out", [LANES, F_OUT], f32, kind="ExternalOutput")
    tdt = u8 if H8 else bf16
    htab_lo = nc.dram_tensor("htab_lo", [LO_CAP, EB], tdt, kind="Internal")
    htab_hi = nc.dram_tensor("htab_hi", [N_ROWS - LO_CAP, EB], tdt,
                             kind="Internal")

    qn = [0]

    def next_q():
        q = qn[0]
        qn[0] = (q + 1) % NQ
        return q

    with tile.TileContext(nc) as tc:
        with tc.tile_pool(name="const", bufs=1) as constp:
            W_ext_sb = constp.tile([P, ROW_USED], bf16)
            nc.sync.dma_start(W_ext_sb[:], W_ext_d.ap())
            W2_sb = constp.tile([P, F_OUT], bf16)
            nc.sync.dma_start(W2_sb[:], W2_d.ap())
            idx_sb = constp.tile([P, cols], i16)
            nc.sync.dma_start(idx_sb[:], idx_d.ap())
            ident = constp.tile([P, P], f32)
            make_identity(nc, ident[:])
            adt_all = constp.tile([P, T_TILES * HEADS], bf16)
            if bias_nz or b2_nz:
                ones_sb = constp.tile([1, P], f32)
                nc.vector.memset(ones_sb[:], 1.0)
            if bias_nz:
                bias_row_sb = constp.tile([1, F_IN], f32)
                nc.sync.dma_start(bias_row_sb[:], bias_d.ap())
                bias_rep = constp.tile([P, F_IN], f32)
            if b2_nz:
                b2_sb = constp.tile([1, F_OUT], f32)
                nc.sync.dma_start(b2_sb[:], b2_d.ap())

            # ---- phase 0: htab rows = [h_cmaj | a_s | a_d] ----
            with (
                tc.tile_pool(name="xfull", bufs=1) as xfp,
                tc.tile_pool(name="hx", bufs=4) as hxp,
                tc.tile_pool(name="p0ps", bufs=6, space="PSUM") as p0ps,
            ):
                NC_ROWS = (P0_CHUNKS * P0_CH * P if sim_safe
                           else n_comp)
                xt = xfp.tile([P, NC_ROWS], bf16)
                NX = 16
                XSPL = NC_ROWS // NX // (P0_CH * P) * (P0_CH * P)
                xbounds = [i * XSPL for i in range(NX)] + [NC_ROWS]
                xengs = [nc.sync, nc.scalar]
                for xi in range(NX):
                    xengs[xi % 2].dma_start(
                        xt[:, xbounds[xi]:xbounds[xi + 1]],
                        x_T.ap()[:, xbounds[xi]:xbounds[xi + 1]])
                if bias_nz:
                    bps = p0ps.tile([P, F_IN], f32)
                    nc.tensor.matmul(bps[:], lhsT=ones_sb[:],
                                     rhs=bias_row_sb[:], start=True, stop=True)
                    nc.vector.tensor_copy(bias_rep[:], bps[:])
                RU = RB_A if H8 else ROW_USED   # table elems used per row
                NTL = NC_ROWS // P               # total node tiles computed
                LO_T = LO_CAP // P               # 256 tiles in the lo table
                ti = 0
                while ti < NTL:
                    seg_end = LO_T if ti < LO_T else NTL
                    nt = min(P0_GRP * P0_CH, seg_end - ti)
                    c0 = ti * P
                    hx = hxp.tile([P, P0_GRP * P0_CH * RU],
                                  u8 if H8 else bf16, tag="hx")
                    hx3 = hx[:, 0:nt * RU].rearrange("p (t e) -> p t e", t=nt)
                    gi = 0
                    while gi < nt:
                        ng = min(P0_CH, nt - gi)
                        ps = p0ps.tile([P, P0_CH * ROW_USED], f32, tag="ps")
                        for j in range(ng):
                            jj = gi + j
                            nc.tensor.matmul(
                                ps[:, j * ROW_USED:(j + 1) * ROW_USED],
                                lhsT=xt[:, c0 + jj * P:c0 + (jj + 1) * P],
                                rhs=W_ext_sb[:], start=True, stop=True)
                        ps3 = (ps[:, 0:ng * ROW_USED]
                               .rearrange("p (t e) -> p t e", t=ng))
                        gs = slice(gi, gi + ng)
                        if H8:
                            nc.scalar.activation(
                                out=hx3[:, gs, 0:F_IN].bitcast(fp8),
                                in_=ps3[:, :, 0:F_IN], func=Copy)
                            nc.scalar.activation(
                                out=hx3[:, gs, F_IN:RB_A].bitcast(bf16),
                                in_=ps3[:, :, F_IN:ROW_USED], func=Copy)
                        elif (gi // P0_CH) % 2 == 0:
                            nc.scalar.activation(
                                out=hx3[:, gs, :], in_=ps3, func=Copy)
                        else:
                            nc.vector.tensor_copy(hx3[:, gs, :], ps3)
                        gi += ng
                    if ti < LO_T:
                        dst = htab_lo.ap()[c0:c0 + nt * P, 0:RU]
                    else:
                        dst = htab_hi.ap()[c0 - LO_CAP:c0 - LO_CAP + nt * P,
                                           0:RU]
                    nc.sync.dma_start(
                        dst.rearrange("(t p) e -> p t e", p=P), hx3)
                    ti += nt
                if sim_safe:
                    # sim's NaN canary: initialize the tables' row padding
                    zpad = xfp.tile([P, EB], u8 if H8 else bf16)
                    nc.gpsimd.memset(zpad[:], 0)
                    for tens, nrows in ((htab_lo, LO_CAP),
                                        (htab_hi, N_ROWS - LO_CAP)):
                        for r0 in range(0, nrows, P):
                            nc.sync.dma_start(
                                tens.ap()[r0:r0 + P, RU:]
                                .rearrange("(t p) e -> p t e", p=P),
                                zpad[:, 0:EB - RU].unsqueeze(1)
                                .to_broadcast([P, 1, EB - RU]))

            # batched a_d for all tiles: adt_all[p, t*4:(t+1)*4]
            if H8:
                ad_src = (htab_lo.ap()[0:LANES, F_IN + 2 * HEADS:RB_A]
                          .bitcast(bf16))
            else:
                ad_src = htab_lo.ap()[0:LANES,
                                      F_IN + HEADS:F_IN + 2 * HEADS]
            nc.sync.dma_start(
                adt_all[:].rearrange("p (t h) -> p t h", t=T_TILES),
                ad_src.rearrange("(t p) h -> p t h", p=P))

            # ---- phase 1: per destination tile ----
            with (
                tc.tile_pool(name="g", bufs=4) as gp,
                tc.tile_pool(name="mb", bufs=2) as mbp,
                tc.tile_pool(name="small", bufs=2) as smallp,
                tc.tile_pool(name="ys", bufs=2) as ysp,
                tc.tile_pool(name="pt", bufs=2, space="PSUM") as ptp,
                tc.tile_pool(name="yp", bufs=2, space="PSUM") as ypp,
            ):
                ysb = None
                for t in range(T_TILES):
                    K = K_lo[t] + K_hi[t]
                    g = gp.tile([P, K * EB], u8 if H8 else bf16, tag="g")
                    g3 = g[:].rearrange("p (k e) -> p k e", k=K)
                    if H8:
                        gv_h = g3[:, :, 0:F_IN].bitcast(fp8)
                        gv_as = (g3[:, :, F_IN:RB_A]
                                 .bitcast(bf16)[:, :, 0:HEADS])
                    else:
                        gv_h = g3[:, :, 0:F_IN]
                        gv_as = g3[:, :, F_IN:F_IN + HEADS]
                    for is_hi, s0, nrows, col0 in sched[t]:
                        nc.gpsimd.dma_gather(
                            out_ap=g3[:, s0:s0 + nrows // P, :],
                            in_ap=(htab_hi.ap() if is_hi
                                   else htab_lo.ap()),
                            idxs_ap=idx_sb[:, col0:col0 + nrows // 16],
                            num_idxs=nrows, num_idxs_reg=nrows,
                            elem_size=EB, queue_num=next_q())

                    exb = smallp.tile([P, K * HEADS], bf16, tag="exb")
                    ex3 = exb[:].rearrange("p (k h) -> p k h", k=K)
                    nc.vector.tensor_tensor(
                        out=ex3, in0=gv_as,
                        in1=adt_all[:, t * HEADS:(t + 1) * HEADS]
                        .unsqueeze(1).to_broadcast([P, K, HEADS]),
                        op=add)
                    nc.vector.scalar_tensor_tensor(
                        out=exb[:], in0=exb[:], scalar=NEG, in1=exb[:],
                        op0=mult, op1=amax)
                    mb = mbp.tile([P, K * 132], bf16, tag="mb")
                    m3 = mb[:].rearrange("p (k f) -> p k f", k=K)
                    nc.scalar.activation(out=m3[:, :, F_IN:132], in_=ex3,
                                         func=Exp)
                    nc.vector.tensor_tensor(
                        out=m3[:, :, 0:F_IN].rearrange(
                            "p k (c h) -> p k c h", h=HEADS),
                        in0=gv_h.rearrange(
                            "p k (c h) -> p k c h", h=HEADS),
                        in1=m3[:, :, F_IN:132].unsqueeze(2).to_broadcast(
                            [P, K, HIDDEN, HEADS]),
                        op=mult)

                    accf = smallp.tile([P, 132], f32, tag="accf")
                    kk = K
                    while kk > 2:
                        pr = kk // 2
                        nc.vector.tensor_tensor(
                            out=mb[:, 0:pr * 132], in0=mb[:, 0:pr * 132],
                            in1=mb[:, (kk - pr) * 132:kk * 132], op=add)
                        kk -= pr
                    if kk == 2:
                        # fold + eps in one op (adding eps to h cols is benign)
                        nc.vector.scalar_tensor_tensor(
                            out=accf[:], in0=mb[:, 0:132], scalar=1e-16,
                            in1=mb[:, 132:264], op0=add, op1=add)
                    else:
                        nc.vector.tensor_scalar_add(out=accf[:],
                                                    in0=mb[:, 0:132],
                                                    scalar1=1e-16)

                    rec = smallp.tile([P, HEADS], f32, tag="rec")
                    nc.vector.reciprocal(rec[:], accf[:, F_IN:132])
                    zn = smallp.tile([P, F_IN], f32, tag="zn")
                    nc.vector.scalar_tensor_tensor(
                        out=zn[:].rearrange("p (c h) -> p c h", h=HEADS),
                        in0=accf[:, 0:F_IN].rearrange("p (c h) -> p c h",
                                                      h=HEADS),
                        scalar=4.0 if H8 else 1.0,
                        in1=rec[:].unsqueeze(1).to_broadcast(
                            [P, HIDDEN, HEADS]),
                        op0=mult, op1=mult)
                    if bias_nz:
                        nc.vector.tensor_tensor(out=zn[:], in0=zn[:],
                                                in1=bias_rep[:], op=add)
                    # ELU(z) = max(z, exp(-Relu(-z)) - 1)
                    tmp = smallp.tile([P, F_IN], f32, tag="tmp")
                    nc.scalar.activation(out=tmp[:], in_=zn[:], func=Relu,
                                         scale=-1.0)
                    nc.scalar.activation(out=tmp[:], in_=tmp[:], func=Exp,
                                         scale=-1.0)
                    nc.vector.scalar_tensor_tensor(
                        out=zn[:], in0=tmp[:], scalar=-1.0, in1=zn[:],
                        op0=add, op1=amax)

                    pt = ptp.tile([P, P], f32, tag="pt")
                    nc.tensor.transpose(out=pt[:], in_=zn[:],
                                        identity=ident[:])
                    znT = smallp.tile([P, P], bf16, tag="znT")
                    nc.scalar.activation(out=znT[:], in_=pt[:], func=Copy)
                    yp = ypp.tile([P, F_OUT], f32, tag="yp")
                    nc.tensor.matmul(yp[:], lhsT=znT[:], rhs=W2_sb[:],
                                     start=True, stop=not b2_nz)
                    if b2_nz:
                        nc.tensor.matmul(yp[:], lhsT=ones_sb[:], rhs=b2_sb[:],
                                         start=False, stop=True)
                    if t % YB == 0:
                        ysb = ysp.tile([P, YB * F_OUT], f32, tag="ysb")
                    nc.scalar.activation(
                        out=ysb[:, (t % YB) * F_OUT:(t % YB + 1) * F_OUT],
                        in_=yp[:], func=Copy)
                    if t % YB == YB - 1:
                        t0 = t - (YB - 1)
                        nc.sync.dma_start(
                            y_d.ap()[t0 * P:(t + 1) * P, :]
                            .rearrange("(t p) f -> p t f", p=P),
                            ysb[:].rearrange("p (t f) -> p t f", t=YB))

    nc.compile()
    return nc


_MODULE_CACHE = {}


def _get_module(K_lo, K_hi, cols, bias_nz, b2_nz, sim_safe, n_comp):
    key = (K_lo, K_hi, cols, bias_nz, b2_nz, sim_safe, H8, n_comp)
    if key not in _MODULE_CACHE:
        _MODULE_CACHE[key] = _build_module(K_lo, K_hi, cols, bias_nz, b2_nz,
                                           sim_safe, n_comp)
    return _MODULE_CACHE[key]


def _ensure_ntff_hook():
    """The axon NTFF profile hook lives in antenv.axon_hooks, which this
    image's antenv package lacks; shim it so trace=True works."""
    try:
        import antenv.axon_hooks  # noqa: F401
        return
    except ImportError:
        pass
    import types

    import antenv

    mod = types.ModuleType("antenv.axon_hooks")
    holder = {"h": None}
    mod.set_axon_ntff_profile_hook = lambda h: holder.__setitem__("h", h)
    mod.get_axon_ntff_profile_hook = lambda: holder["h"]
    try:
        from trn_agent_boot.trn_boot import _ntff_profile_via_ctypes
        holder["h"] = _ntff_profile_via_ctypes("/opt/axon/libaxon_pjrt.so")
    except Exception:
        pass
    sys.modules["antenv.axon_hooks"] = mod
    antenv.axon_hooks = mod


def kernel(x, edge_index, edge_weight, W, a_src, a_dst, bias, W2, b2,
           _trace=False, _sim=False):
    from concourse.bass_utils import run_bass_kernel_spmd

    if _trace:
        _ensure_ntff_hook()

    x = np.ascontiguousarray(np.asarray(x, np.float32))
    W = np.asarray(W, np.float32)
    a_src = np.asarray(a_src, np.float32)
    a_dst = np.asarray(a_dst, np.float32)
    bias = np.asarray(bias, np.float32)
    W2 = np.ascontiguousarray(np.asarray(W2, np.float32))
    b2 = np.asarray(b2, np.float32)

    W_ext, cores, K_lo, K_hi, cols, n_comp = _prep(x, edge_index, W, a_src,
                                                   a_dst)
    bias_nz = bool(np.any(bias))
    b2_nz = bool(np.any(b2))
    nc = _get_module(K_lo, K_hi, cols, bias_nz, b2_nz, _sim, n_comp)

    bf = ml_dtypes.bfloat16
    W_ext_bf = W_ext.astype(bf)
    W2_bf = W2[PERM_CMAJ, :].astype(bf)

    in_maps = []
    for cr in cores:
        m = {
            "x_T": cr["xT"].astype(bf),
            "W_ext": W_ext_bf,
            "W2": W2_bf,
            "idx": cr["idx"],
        }
        if bias_nz:
            m["bias_row"] = np.ascontiguousarray(
                bias[PERM_CMAJ].reshape(1, F_IN))
        if b2_nz:
            m["b2_row"] = b2.reshape(1, F_OUT)
        in_maps.append(m)

    if _sim:
        from concourse.bass_interp import CoreSim
        sim = CoreSim(nc, trace=False)
        for k, v in in_maps[0].items():
            sim.tensor(k)[:] = v
        sim.simulate()
        y0 = np.asarray(sim.tensor("y_out"), np.float32)
        out = np.zeros((N_NODES, F_OUT), np.float32)
        out[cores[0]["own_sorted"]] = y0[:NPC]
        kernel.sim_out = out
        return out

    res = run_bass_kernel_spmd(nc, in_maps, core_ids=list(range(N_CORES)),
                               trace=_trace)
    out = np.zeros((N_NODES, F_OUT), np.float32)
    for c, cr in enumerate(cores):
        out[cr["own_sorted"]] = np.asarray(
            res.results[c]["y_out"], np.float32)[:NPC]
    if _trace:
        kernel.last_results = res
    return out
